# revision 32
# baseline (speedup 1.0000x reference)
"""Multi-head causal attention forward on 8 Trainium2 NeuronCores.

Reference computation (B=2, S=2048, D=1024, H=16, Dh=64):
    q/k/v = einsum("bsm,hmd->bshd", x, W_{Q,K,V}) (+ zero biases)
    scores = q @ k^T / sqrt(Dh), causal mask, softmax
    z = attn @ v
    out = einsum("bqhd,hdm->bqm", z, W_O) + sum_h b_O[h]

Sharding: core c handles batch c//4 and heads 4*(c%4) .. 4*(c%4)+3
(tensor parallel over heads x data parallel over batch). Each core
produces a partial output (sum over its 4 heads); the host sums the 4
partials per batch (the "all-reduce" of the output projection).

v2 structure (vs v1): k-blocks processed in groups of 4 (scores run,
then z runs per head) so the PE streams at full rate; out-projection
shares the score psum ring; normalize reads PSUM directly (DVE
reciprocal + gpsimd broadcast, no ACT log/exp chain and no staging
copy); v' evacuated with one strided copy per s-block; output stored
bf16 (host casts); projections interleaved into ACT-bound slack.
"""

import os
import sys

import numpy as np

if "/opt/trn_rl_repo" not in sys.path:
    sys.path.insert(0, "/opt/trn_rl_repo")

import concourse.bass as bass
import concourse.bacc as bacc
import concourse.tile as tile
from concourse import mybir
from concourse.bass_utils import run_bass_kernel_spmd

B, S, D, H, Dh = 2, 2048, 1024, 16, 64
HPC = 4          # heads per core
N_CORES = 8
QCH = 512        # q chunk width (one psum bank of fp32)
F32 = mybir.dt.float32
BF16 = mybir.dt.bfloat16


def _build_masks() -> np.ndarray:
    """Triangular band mask [128, 128]: (kp, q) valid iff kp <= q."""
    tri = (np.arange(128)[None, :] >= np.arange(128)[:, None])
    import ml_dtypes
    return np.ascontiguousarray(tri.astype(ml_dtypes.bfloat16))


def build_bass() -> bass.Bass:
    nc = bacc.Bacc("TRN2", target_bir_lowering=False, debug=False)

    xt_d = nc.dram_tensor("xt", [D, S], BF16, kind="ExternalInput")
    wq_d = nc.dram_tensor("wq", [2, D, 128], BF16, kind="ExternalInput")
    wk_d = nc.dram_tensor("wk", [2, D, 128], BF16, kind="ExternalInput")
    wv_d = nc.dram_tensor("wv", [D, HPC * Dh], BF16, kind="ExternalInput")
    wo_d = nc.dram_tensor("wo", [2, 128, D], BF16, kind="ExternalInput")
    out_d = nc.dram_tensor("out", [S, D], BF16, kind="ExternalOutput")
    mask_d = nc.inline_tensor(_build_masks(), "cmask")

    xt = xt_d.ap()
    wq = wq_d.ap()
    wk = wk_d.ap()
    wv = wv_d.ap()
    wo = wo_d.ap()
    out = out_d.ap()
    mask = mask_d.ap()

    EXP = mybir.ActivationFunctionType.Exp

    with tile.TileContext(nc) as tc:
        const_pool = tc.alloc_tile_pool(name="const", bufs=1)
        persist = tc.alloc_tile_pool(name="persist", bufs=1)
        psum_s = tc.alloc_tile_pool(name="psum_s", bufs=3, space="PSUM")
        psum_z = tc.alloc_tile_pool(name="psum_z", bufs=2, space="PSUM")

        mask_sb = const_pool.tile([128, 128], BF16, name="mask_sb")

        wo_sb = [persist.tile([128, D], BF16, name=f"wo_sb{p}") for p in range(2)]
        qT = [persist.tile([128, S], BF16, name=f"qT{p}") for p in range(2)]
        kT = [persist.tile([128, S], BF16, name=f"kT{p}") for p in range(2)]
        # v' packed: [128, head, sblock, 64+ones]
        vp = persist.tile([128, HPC, 16, 65], BF16, name="vp")
        zT = [
            [persist.tile([128, QCH], BF16, name=f"zT{p}_{qc}") for qc in range(4)]
            for p in range(2)
        ]

        proj = tc.alloc_tile_pool(name="proj", bufs=1)
        wq_sb, wk_sb = [], []
        for p in range(2):
            wq_sb.append(proj.tile([128, 8, 128], BF16, name=f"wq_sb{p}"))
            wk_sb.append(proj.tile([128, 8, 128], BF16, name=f"wk_sb{p}"))
        wv_sb = proj.tile([128, 8, HPC * Dh], BF16, name="wv_sb")
        xt_sb = [proj.tile([128, S], BF16, name=f"xt_sb{m}") for m in range(8)]

        # DMA order: first projection chain needs wk pair0 + xt chunks, the
        # warmup covers the latency.
        nc.sync.dma_start(out=wk_sb[0], in_=wk[0].rearrange("(c p) d -> p c d", p=128))
        nc.sync.dma_start(out=wq_sb[0], in_=wq[0].rearrange("(c p) d -> p c d", p=128))
        nc.sync.dma_start(out=wv_sb, in_=wv.rearrange("(c p) d -> p c d", p=128))
        # first s-quarter of every xt chunk first (unblocks proj ci0 and v
        # sb0-3 after ~1MB), then the pair-1 weights (needed by ~25us for
        # (0,p1)'s projections), then the xt remainder
        for m in range(8):
            nc.sync.dma_start(out=xt_sb[m][:, 0:QCH], in_=xt[m * 128 : (m + 1) * 128, 0:QCH])
        nc.sync.dma_start(out=wk_sb[1], in_=wk[1].rearrange("(c p) d -> p c d", p=128))
        nc.sync.dma_start(out=wq_sb[1], in_=wq[1].rearrange("(c p) d -> p c d", p=128))
        for m in range(8):
            nc.sync.dma_start(out=xt_sb[m][:, QCH:S], in_=xt[m * 128 : (m + 1) * 128, QCH:S])
        for p in range(2):
            nc.sync.dma_start(out=wo_sb[p], in_=wo[p])
        nc.sync.dma_start(out=mask_sb, in_=mask)

        # HAM warmup: dummy matmuls with no DMA dependencies fill the input
        # DMA wait and push the PE activity monitor to the full clock.
        warm = proj.tile([128, QCH], BF16, name="warm")
        nc.vector.memset(warm, 1.0)
        for i in range(22):
            wps = psum_s.tile([128, 2 * QCH], F32, tag="s", name=f"wps{i}")[:, :QCH]
            nc.tensor.matmul(
                wps, lhsT=warm[:, 0:128], rhs=warm, start=True, stop=True
            )

        # ones columns of v' (1.0 in bf16 exact)
        nc.vector.memset(vp[:, :, :, 64], 1.0)

        # ---- projection chunklets (emitted lazily into attention slack) ----
        def emit_qk_ci(p, which, ci):
            w_sb = wq_sb[p] if which == "q" else wk_sb[p]
            dst = qT[p] if which == "q" else kT[p]
            ps = psum_s.tile([128, 2 * QCH], F32, tag="s", name=f"pqk{p}{which}{ci}")[:, :QCH]
            for mc in range(8):
                nc.tensor.matmul(
                    ps,
                    lhsT=w_sb[:, mc, :],
                    rhs=xt_sb[mc][:, ci * QCH : (ci + 1) * QCH],
                    start=(mc == 0),
                    stop=(mc == 7),
                )
            nc.vector.tensor_copy(dst[:, ci * QCH : (ci + 1) * QCH], ps)

        def emit_v_sb(sb):
            psv = psum_s.tile([128, 2 * QCH], F32, tag="s", name=f"psv{sb}")[:, : HPC * Dh]
            for mc in range(8):
                nc.tensor.matmul(
                    psv,
                    lhsT=xt_sb[mc][:, sb * 128 : (sb + 1) * 128],
                    rhs=wv_sb[:, mc, :],
                    start=(mc == 0),
                    stop=(mc == 7),
                )
            # one strided copy: psv [128, 4*64] -> vp[:, h, sb, 0:64]
            nc.vector.tensor_copy(vp[:, :, sb, 0:64], psv.rearrange("p (h d) -> p h d", h=HPC))

        # Work queue of projection chunklets in dependency-useful order.
        # Prologue (emitted before attention): k0c0, q0c0, v0..v3.
        # Gates (see need_idx) force-drain everything a (qc, pair) block
        # requires; emit_chunklets(1) opportunistically runs ahead of them.
        chunk_queue = [
            ("qk", 1, "k", 0), ("qk", 1, "q", 0),          # 0-1: for (0, p1)
            ("qk", 0, "k", 1), ("qk", 0, "q", 1),          # 2-3: for (1, p0)
            ("v", 4), ("v", 5), ("v", 6), ("v", 7),        # 4-7: for qc1 z
            ("qk", 1, "k", 1), ("qk", 1, "q", 1),          # 8-9: for (1, p1)
            ("qk", 0, "k", 2), ("qk", 0, "q", 2),          # 10-11: for (2, p0)
            ("v", 8), ("v", 9), ("v", 10), ("v", 11),      # 12-15: for qc2 z
            ("qk", 1, "k", 2), ("qk", 1, "q", 2),          # 16-17: for (2, p1)
            ("qk", 0, "k", 3), ("qk", 0, "q", 3),          # 18-19: for (3, p0)
            ("v", 12), ("v", 13), ("v", 14), ("v", 15),    # 20-23: for qc3 z
            ("qk", 1, "k", 3), ("qk", 1, "q", 3),          # 24-25: for (3, p1)
        ]
        # minimum chunklets that must be emitted before (qc, pair) starts
        need_idx = {
            (0, 0): 0, (0, 1): 2,
            (1, 0): 8, (1, 1): 10,
            (2, 0): 16, (2, 1): 18,
            (3, 0): 24, (3, 1): 26,
        }
        emitted_chunks = [0]

        def _emit_one():
            c = chunk_queue[emitted_chunks[0]]
            emitted_chunks[0] += 1
            if c[0] == "qk":
                emit_qk_ci(c[1], c[2], c[3])
            elif c[0] == "proj":
                emit_proj_qb(c[1], c[2])
            else:
                emit_v_sb(c[1])

        def emit_chunklets(n):
            for _ in range(n):
                if emitted_chunks[0] >= len(chunk_queue):
                    return
                _emit_one()

        def drain_to(n):
            while emitted_chunks[0] < n:
                _emit_one()

        # prologue projections
        emit_qk_ci(0, "k", 0)
        emit_qk_ci(0, "q", 0)
        for sb in range(4):
            emit_v_sb(sb)

        pt_pool = tc.alloc_tile_pool(name="pt", bufs=10)
        small = tc.alloc_tile_pool(name="small", bufs=6)
        ost = tc.alloc_tile_pool(name="ost", bufs=3)

        # ---- output projection, one 128-row block ----
        def emit_proj_qb(qc, qi):
                qb = 4 * qc + qi
                pso = psum_s.tile([128, 2 * QCH], F32, tag="s", name=f"pso{qb}")
                for mc in range(2):
                    for p in range(2):
                        nc.tensor.matmul(
                            pso[:, mc * QCH : (mc + 1) * QCH],
                            lhsT=zT[p][qc][:, qi * 128 : (qi + 1) * 128],
                            rhs=wo_sb[p][:, mc * QCH : (mc + 1) * QCH],
                            start=(p == 0),
                            stop=(p == 1),
                        )
                outsb = ost.tile([128, D], BF16, tag="ost", name=f"outsb{qb}")
                nc.vector.tensor_copy(outsb, pso)
                if qb < 12:
                    nc.sync.dma_start(out=out[qb * 128 : qb * 128 + 128, :], in_=outsb)
                else:
                    nc.sync.dma_start(
                        out=out[qb * 128 : qb * 128 + 128, 0:QCH], in_=outsb[:, 0:QCH]
                    )
                    nc.sync.dma_start(
                        out=out[qb * 128 : qb * 128 + 128, QCH:D], in_=outsb[:, QCH:D]
                    )

        def emit_proj(qc):
            for qi in range(4):
                emit_proj_qb(qc, qi)

        # out-proj blocks of qc 0..2 ride the filler queue: consumed in qc3
        # (gates keep them after all projection chunklets; zT of qc<=2 is
        # normalized before qc3 starts)
        for qc_ in range(3):
            for qi_ in range(4):
                chunk_queue.append(("proj", qc_, qi_))

        # ---- deferred normalize: psz -> zT (recip + broadcast + mul) ----
        def flush_normalize(item):
            psz, pair_, qc_ = item
            for hh in (0, 1):
                den0 = small.tile([1, QCH], F32, tag="den0", name=f"dn{pair_}{qc_}{hh}")
                nc.vector.tensor_copy(den0, psz[hh][64:65, :])
                rden = small.tile([1, QCH], F32, tag="rden", name=f"rd{pair_}{qc_}{hh}")
                nc.vector.reciprocal_approx_fast(rden, den0)
                rb = small.tile([64, QCH], F32, tag="rb", name=f"rb{pair_}{qc_}{hh}")
                nc.gpsimd.partition_broadcast(rb, rden)
                nc.vector.tensor_mul(
                    zT[pair_][qc_][hh * 64 : (hh + 1) * 64, :],
                    psz[hh][0:64, :],
                    rb,
                )

        # ---- attention main loop ----
        # zq: z work of the previous group, lagged by one group ACROSS pair
        # boundaries; emitting it right after each group's scores pads the
        # PE stream with exp-independent work so scores never catch the exp
        # spine. When the lagged group was its pair's last, that pair's psz
        # is complete afterwards and is normalized immediately.
        zq = None

        def emit_zq():
            zpsz, zpair, zqc, znkb, items, last = zq
            for hh in (0, 1):
                for kb, (pss_, rel_, dt2_, pt_) in items:
                    nc.tensor.matmul(
                        zpsz[hh][:, rel_:QCH],
                        lhsT=vp[:, 2 * zpair + hh, kb, :],
                        rhs=pt_[:, hh * QCH + rel_ : (hh + 1) * QCH],
                        start=(kb == 0),
                        stop=(kb == znkb - 1),
                    )
            if last:
                flush_normalize((zpsz, zpair, zqc))

        for qc in range(4):
            n_kb = 4 * qc + 4
            for pair in range(2):
                drain_to(need_idx[(qc, pair)])
                psz = [
                    psum_z.tile([65, QCH], F32, tag="z", name=f"psz{pair}{qc}{hh}")
                    for hh in (0, 1)
                ]
                for g in range(n_kb // 4):
                    kbs = range(4 * g, 4 * g + 4)
                    pss_g = {}
                    for kb in kbs:
                        dt2 = kb - (n_kb - 4)
                        rel = max(dt2, 0) * 128
                        pss = psum_s.tile(
                            [128, 2 * QCH], F32, tag="s", name=f"pss{pair}{qc}{kb}"
                        )
                        pss_g[kb] = (pss, rel, dt2)
                        for hh in (0, 1):
                            hoff = hh * 64
                            nc.tensor.matmul(
                                pss[:, hh * QCH + rel : (hh + 1) * QCH],
                                lhsT=kT[pair][hoff : hoff + 64, kb * 128 : (kb + 1) * 128],
                                rhs=qT[pair][
                                    hoff : hoff + 64, qc * QCH + rel : (qc + 1) * QCH
                                ],
                                start=True,
                                stop=True,
                                tile_position=(hoff, 0),
                            )
                        pt = pt_pool.tile(
                            [128, 2 * QCH], BF16, tag="pt", name=f"pt{pair}{qc}{kb}"
                        )
                        pss_g[kb] += (pt,)
                        if rel >= 256:
                            for hh in (0, 1):
                                off = hh * QCH + rel
                                nc.scalar.activation(
                                    pt[:, off : hh * QCH + QCH],
                                    pss[:, off : hh * QCH + QCH],
                                    EXP,
                                    scale=0.125,
                                )
                        else:
                            nc.scalar.activation(pt, pss, EXP, scale=0.125)
                        if dt2 >= 0:
                            for hh in (0, 1):
                                off = hh * QCH + rel
                                nc.vector.tensor_mul(
                                    pt[:, off : off + 128], pt[:, off : off + 128], mask_sb
                                )
                    if zq is not None:
                        emit_zq()
                    zq = (psz, pair, qc, n_kb,
                          [(kb, pss_g[kb]) for kb in kbs],
                          g == n_kb // 4 - 1)
                    # interleave a filler chunklet into the exp-bound slack
                    emit_chunklets(1)
                # out-projections deferred into qc3 (PE idle under its exp
                # load): remaining proj(0)/(1) blocks drain at (3,0) end,
                # proj(2) blocks fill (3,1)'s groups
                if qc == 3 and pair == 0:
                    drain_to(34)
        drain_to(len(chunk_queue))
        emit_zq()  # (3,1)'s final group + its normalize
        emit_proj(3)

        ost.release()
        small.release()
        pt_pool.release()
        proj.release()
        psum_z.release()
        psum_s.release()
        persist.release()
        const_pool.release()

    nc.compile()
    return nc


_NC_CACHE: list = []


def _get_nc() -> bass.Bass:
    if not _NC_CACHE:
        _NC_CACHE.append(build_bass())
    return _NC_CACHE[0]


def _core_inputs(x, W_Q, W_K, W_V, W_O, c):
    b = c // HPC
    h0 = HPC * (c % HPC)
    wq = np.stack(
        [W_Q[h0 + 2 * p : h0 + 2 * p + 2].transpose(1, 0, 2).reshape(D, 128) for p in range(2)]
    )
    wk = np.stack(
        [W_K[h0 + 2 * p : h0 + 2 * p + 2].transpose(1, 0, 2).reshape(D, 128) for p in range(2)]
    )
    wv = W_V[h0 : h0 + HPC].transpose(1, 0, 2).reshape(D, HPC * Dh)
    wo = np.stack([W_O[h0 + 2 * p : h0 + 2 * p + 2].reshape(128, D) for p in range(2)])
    import ml_dtypes

    bf = ml_dtypes.bfloat16
    return {
        "xt": np.ascontiguousarray(x[b].T.astype(bf)),
        "wq": np.ascontiguousarray(wq.astype(bf)),
        "wk": np.ascontiguousarray(wk.astype(bf)),
        "wv": np.ascontiguousarray(wv.astype(bf)),
        "wo": np.ascontiguousarray(wo.astype(bf)),
    }


def _ensure_ntff_hook():
    """Install the axon NTFF profile hook if the image's antenv lacks it.

    Only needed for trace=True runs (test harness); the grading path
    (kernel()) never calls this.
    """
    try:
        from antenv.axon_hooks import get_axon_ntff_profile_hook  # noqa: F401
        return
    except ImportError:
        pass
    import types

    import antenv

    holder = {"hook": None}
    mod = types.ModuleType("antenv.axon_hooks")
    mod.set_axon_ntff_profile_hook = lambda h: holder.__setitem__("hook", h)
    mod.get_axon_ntff_profile_hook = lambda: holder["hook"]
    sys.modules["antenv.axon_hooks"] = mod
    antenv.axon_hooks = mod
    try:
        if "/root/.axon_site" not in sys.path:
            sys.path.insert(0, "/root/.axon_site")
        from trn_agent_boot.trn_boot import _ntff_profile_via_ctypes

        so = "/opt/axon/libaxon_pjrt.so"
        if os.path.exists(so):
            mod.set_axon_ntff_profile_hook(_ntff_profile_via_ctypes(so))
    except Exception as e:  # degrade to no tracing
        print(f"NTFF hook install failed: {e}", file=sys.stderr)
    # artifact upload needs S3 creds this container may not have
    import concourse.bass_utils as bu

    bu.upload_artifacts = lambda tmpdir: f"local://{tmpdir}"


def _run(inputs: dict, trace: bool = False):
    x = np.asarray(inputs["x"], np.float32)
    W_Q = np.asarray(inputs["W_Q"], np.float32)
    W_K = np.asarray(inputs["W_K"], np.float32)
    W_V = np.asarray(inputs["W_V"], np.float32)
    W_O = np.asarray(inputs["W_O"], np.float32)
    b_O = np.asarray(inputs["b_O"], np.float32)

    if trace:
        _ensure_ntff_hook()
    nc = _get_nc()
    in_maps = [_core_inputs(x, W_Q, W_K, W_V, W_O, c) for c in range(N_CORES)]
    res = run_bass_kernel_spmd(nc, in_maps, core_ids=list(range(N_CORES)), trace=trace)

    out = np.zeros((B, S, D), np.float32)
    for c in range(N_CORES):
        out[c // HPC] += np.asarray(res.results[c]["out"], np.float32)
    out += b_O.sum(axis=0)  # b_O is [H, D]; reference adds sum over heads
    return out, res


def kernel(**inputs) -> np.ndarray:
    # b_Q/b_K/b_V are zero in the reference's setup_inputs; the device
    # kernel folds them out. Guard with an exact fallback just in case.
    for name in ("b_Q", "b_K", "b_V"):
        if name in inputs and np.any(np.asarray(inputs[name])):
            return _kernel_numpy_fallback(**inputs)
    out, _ = _run(inputs)
    if not np.isfinite(out).all():
        # transient device flake (observed rarely); one retry clears it
        out, _ = _run(inputs)
    return out


def _kernel_numpy_fallback(x, W_Q, b_Q, W_K, b_K, W_V, W_O, b_V, b_O):
    x = np.asarray(x, np.float32)
    q = np.einsum("bqm,hmd->bqhd", x, W_Q) + b_Q
    k = np.einsum("bkm,hmd->bkhd", x, W_K) + b_K
    v = np.einsum("bkm,hmd->bkhd", x, W_V) + b_V
    s = np.einsum("bqhd,bkhd->bhqk", q, k) / np.sqrt(np.float32(W_Q.shape[-1]))
    causal = np.tril(np.ones((x.shape[1], x.shape[1]), bool))
    s = np.where(causal, s, np.float32(-1e9))
    s = s - s.max(-1, keepdims=True)
    e = np.exp(s)
    attn = e / e.sum(-1, keepdims=True)
    z = np.einsum("bhqk,bkhd->bqhd", attn, v)
    return np.einsum("bqhd,hdm->bqm", z, W_O) + b_O.sum(0)


# revision 33
# speedup vs baseline: 1.1482x; 1.1482x over previous
"""Multi-head causal attention forward on 8 Trainium2 NeuronCores.

Reference computation (B=2, S=2048, D=1024, H=16, Dh=64):
    q/k/v = einsum("bsm,hmd->bshd", x, W_{Q,K,V}) (+ zero biases)
    scores = q @ k^T / sqrt(Dh), causal mask, softmax
    z = attn @ v
    out = einsum("bqhd,hdm->bqm", z, W_O) + sum_h b_O[h]

Sharding: core c handles batch c//4 and heads 4*(c%4) .. 4*(c%4)+3
(tensor parallel over heads x data parallel over batch). Each core
produces a partial output (sum over its 4 heads); the host sums the 4
partials per batch (the "all-reduce" of the output projection).

v2 structure (vs v1): k-blocks processed in groups of 4 (scores run,
then z runs per head) so the PE streams at full rate; out-projection
shares the score psum ring; normalize reads PSUM directly (DVE
reciprocal + gpsimd broadcast, no ACT log/exp chain and no staging
copy); v' evacuated with one strided copy per s-block; output stored
bf16 (host casts); projections interleaved into ACT-bound slack.
"""

import os
import sys

import numpy as np

if "/opt/trn_rl_repo" not in sys.path:
    sys.path.insert(0, "/opt/trn_rl_repo")

import concourse.bass as bass
import concourse.bacc as bacc
import concourse.tile as tile
from concourse import mybir
from concourse.bass_utils import run_bass_kernel_spmd

B, S, D, H, Dh = 2, 2048, 1024, 16, 64
HPC = 4          # heads per core
N_CORES = 8
QCH = 512        # q chunk width (one psum bank of fp32)
F32 = mybir.dt.float32
BF16 = mybir.dt.bfloat16


def _build_masks() -> np.ndarray:
    """Triangular band mask [128, 128]: (kp, q) valid iff kp <= q."""
    tri = (np.arange(128)[None, :] >= np.arange(128)[:, None])
    import ml_dtypes
    return np.ascontiguousarray(tri.astype(ml_dtypes.bfloat16))


def build_bass() -> bass.Bass:
    nc = bacc.Bacc("TRN2", target_bir_lowering=False, debug=False)

    xt_d = nc.dram_tensor("xt", [D, S], BF16, kind="ExternalInput")
    wq_d = nc.dram_tensor("wq", [2, D, 128], BF16, kind="ExternalInput")
    wk_d = nc.dram_tensor("wk", [2, D, 128], BF16, kind="ExternalInput")
    wv_d = nc.dram_tensor("wv", [D, HPC * Dh], BF16, kind="ExternalInput")
    wo_d = nc.dram_tensor("wo", [2, 128, D], BF16, kind="ExternalInput")
    out_d = nc.dram_tensor("out", [S, D], BF16, kind="ExternalOutput")
    mask_d = nc.inline_tensor(_build_masks(), "cmask")

    xt = xt_d.ap()
    wq = wq_d.ap()
    wk = wk_d.ap()
    wv = wv_d.ap()
    wo = wo_d.ap()
    out = out_d.ap()
    mask = mask_d.ap()

    EXP = mybir.ActivationFunctionType.Exp

    with tile.TileContext(nc) as tc:
        const_pool = tc.alloc_tile_pool(name="const", bufs=1)
        persist = tc.alloc_tile_pool(name="persist", bufs=1)
        psum_s = tc.alloc_tile_pool(name="psum_s", bufs=3, space="PSUM")
        psum_z = tc.alloc_tile_pool(name="psum_z", bufs=2, space="PSUM")

        mask_sb = const_pool.tile([128, 128], BF16, name="mask_sb")

        wo_sb = [persist.tile([128, D], BF16, name=f"wo_sb{p}") for p in range(2)]
        qT = [persist.tile([128, S], BF16, name=f"qT{p}") for p in range(2)]
        kT = [persist.tile([128, S], BF16, name=f"kT{p}") for p in range(2)]
        # v' packed: [128, head, sblock, 64+ones]
        vp = persist.tile([128, HPC, 16, 65], BF16, name="vp")
        zT = [
            [persist.tile([128, QCH], BF16, name=f"zT{p}_{qc}") for qc in range(4)]
            for p in range(2)
        ]

        proj = tc.alloc_tile_pool(name="proj", bufs=1)
        wq_sb, wk_sb = [], []
        for p in range(2):
            wq_sb.append(proj.tile([128, 8, 128], BF16, name=f"wq_sb{p}"))
            wk_sb.append(proj.tile([128, 8, 128], BF16, name=f"wk_sb{p}"))
        wv_sb = proj.tile([128, 8, HPC * Dh], BF16, name="wv_sb")
        xt_sb = [proj.tile([128, S], BF16, name=f"xt_sb{m}") for m in range(8)]

        # DMA order: first projection chain needs wk pair0 + xt chunks, the
        # warmup covers the latency.
        nc.sync.dma_start(out=wk_sb[0], in_=wk[0].rearrange("(c p) d -> p c d", p=128))
        nc.sync.dma_start(out=wq_sb[0], in_=wq[0].rearrange("(c p) d -> p c d", p=128))
        nc.sync.dma_start(out=wv_sb, in_=wv.rearrange("(c p) d -> p c d", p=128))
        # first s-quarter of every xt chunk first (unblocks proj ci0 and v
        # sb0-3 after ~1MB), then the pair-1 weights (needed by ~25us for
        # (0,p1)'s projections), then the xt remainder
        for m in range(8):
            nc.sync.dma_start(out=xt_sb[m][:, 0:QCH], in_=xt[m * 128 : (m + 1) * 128, 0:QCH])
        nc.sync.dma_start(out=wk_sb[1], in_=wk[1].rearrange("(c p) d -> p c d", p=128))
        nc.sync.dma_start(out=wq_sb[1], in_=wq[1].rearrange("(c p) d -> p c d", p=128))
        for m in range(8):
            nc.sync.dma_start(out=xt_sb[m][:, QCH:S], in_=xt[m * 128 : (m + 1) * 128, QCH:S])
        for p in range(2):
            nc.sync.dma_start(out=wo_sb[p], in_=wo[p])
        nc.sync.dma_start(out=mask_sb, in_=mask)

        # HAM warmup: dummy matmuls with no DMA dependencies fill the input
        # DMA wait and push the PE activity monitor to the full clock.
        warm = proj.tile([128, QCH], BF16, name="warm")
        nc.vector.memset(warm, 1.0)
        for i in range(22):
            wps = psum_s.tile([128, 2 * QCH], F32, tag="s", name=f"wps{i}")[:, :QCH]
            nc.tensor.matmul(
                wps, lhsT=warm[:, 0:128], rhs=warm, start=True, stop=True
            )

        # ones columns of v' (1.0 in bf16 exact)
        nc.vector.memset(vp[:, :, :, 64], 1.0)

        # ---- projection chunklets (emitted lazily into attention slack) ----
        def emit_qk_ci(p, which, ci):
            w_sb = wq_sb[p] if which == "q" else wk_sb[p]
            dst = qT[p] if which == "q" else kT[p]
            ps = psum_s.tile([128, 2 * QCH], F32, tag="s", name=f"pqk{p}{which}{ci}")[:, :QCH]
            for mc in range(8):
                nc.tensor.matmul(
                    ps,
                    lhsT=w_sb[:, mc, :],
                    rhs=xt_sb[mc][:, ci * QCH : (ci + 1) * QCH],
                    start=(mc == 0),
                    stop=(mc == 7),
                )
            nc.vector.tensor_copy(dst[:, ci * QCH : (ci + 1) * QCH], ps)

        def emit_v_sb(sb):
            psv = psum_s.tile([128, 2 * QCH], F32, tag="s", name=f"psv{sb}")[:, : HPC * Dh]
            for mc in range(8):
                nc.tensor.matmul(
                    psv,
                    lhsT=xt_sb[mc][:, sb * 128 : (sb + 1) * 128],
                    rhs=wv_sb[:, mc, :],
                    start=(mc == 0),
                    stop=(mc == 7),
                )
            # one strided copy: psv [128, 4*64] -> vp[:, h, sb, 0:64]
            nc.vector.tensor_copy(vp[:, :, sb, 0:64], psv.rearrange("p (h d) -> p h d", h=HPC))

        # Work queue of projection chunklets in dependency-useful order.
        # Prologue (emitted before attention): k0c0, q0c0, v0..v3.
        # Gates (see need_idx) force-drain everything a (qc, pair) block
        # requires; emit_chunklets(1) opportunistically runs ahead of them.
        chunk_queue = [
            ("qk", 1, "k", 0), ("qk", 1, "q", 0),          # 0-1: for (0, p1)
            ("qk", 0, "k", 1), ("qk", 0, "q", 1),          # 2-3: for (1, p0)
            ("v", 4), ("v", 5), ("v", 6), ("v", 7),        # 4-7: for qc1 z
            ("qk", 1, "k", 1), ("qk", 1, "q", 1),          # 8-9: for (1, p1)
            ("qk", 0, "k", 2), ("qk", 0, "q", 2),          # 10-11: for (2, p0)
            ("v", 8), ("v", 9), ("v", 10), ("v", 11),      # 12-15: for qc2 z
            ("qk", 1, "k", 2), ("qk", 1, "q", 2),          # 16-17: for (2, p1)
            ("qk", 0, "k", 3), ("qk", 0, "q", 3),          # 18-19: for (3, p0)
            ("v", 12), ("v", 13), ("v", 14), ("v", 15),    # 20-23: for qc3 z
            ("qk", 1, "k", 3), ("qk", 1, "q", 3),          # 24-25: for (3, p1)
        ]
        # minimum chunklets that must be emitted before (qc, pair) starts
        need_idx = {
            (0, 0): 0, (0, 1): 2,
            (1, 0): 8, (1, 1): 10,
            (2, 0): 16, (2, 1): 18,
            (3, 0): 24, (3, 1): 26,
        }
        emitted_chunks = [0]

        def _emit_one():
            c = chunk_queue[emitted_chunks[0]]
            emitted_chunks[0] += 1
            if c[0] == "qk":
                emit_qk_ci(c[1], c[2], c[3])
            elif c[0] == "proj":
                emit_proj_qb(c[1], c[2])
            else:
                emit_v_sb(c[1])

        def emit_chunklets(n):
            for _ in range(n):
                if emitted_chunks[0] >= len(chunk_queue):
                    return
                _emit_one()

        def drain_to(n):
            while emitted_chunks[0] < n:
                _emit_one()

        # prologue projections
        emit_qk_ci(0, "k", 0)
        emit_qk_ci(0, "q", 0)
        for sb in range(4):
            emit_v_sb(sb)

        pt_pool = tc.alloc_tile_pool(name="pt", bufs=10)
        small = tc.alloc_tile_pool(name="small", bufs=6)
        ost = tc.alloc_tile_pool(name="ost", bufs=3)

        # ---- output projection, one 128-row block ----
        def emit_proj_qb(qc, qi):
                qb = 4 * qc + qi
                pso = psum_s.tile([128, 2 * QCH], F32, tag="s", name=f"pso{qb}")
                for mc in range(2):
                    for p in range(2):
                        nc.tensor.matmul(
                            pso[:, mc * QCH : (mc + 1) * QCH],
                            lhsT=zT[p][qc][:, qi * 128 : (qi + 1) * 128],
                            rhs=wo_sb[p][:, mc * QCH : (mc + 1) * QCH],
                            start=(p == 0),
                            stop=(p == 1),
                        )
                outsb = ost.tile([128, D], BF16, tag="ost", name=f"outsb{qb}")
                nc.vector.tensor_copy(outsb, pso)
                if qb < 12:
                    nc.sync.dma_start(out=out[qb * 128 : qb * 128 + 128, :], in_=outsb)
                else:
                    nc.sync.dma_start(
                        out=out[qb * 128 : qb * 128 + 128, 0:QCH], in_=outsb[:, 0:QCH]
                    )
                    nc.sync.dma_start(
                        out=out[qb * 128 : qb * 128 + 128, QCH:D], in_=outsb[:, QCH:D]
                    )

        def emit_proj(qc):
            for qi in range(4):
                emit_proj_qb(qc, qi)

        # out-proj blocks of qc 0..2 ride the filler queue: consumed in qc3
        # (gates keep them after all projection chunklets; zT of qc<=2 is
        # normalized before qc3 starts)
        for qc_ in range(3):
            for qi_ in range(4):
                chunk_queue.append(("proj", qc_, qi_))

        # ---- deferred normalize: psz -> zT (recip + broadcast + mul) ----
        def flush_normalize(item):
            psz, pair_, qc_ = item
            for hh in (0, 1):
                den0 = small.tile([1, QCH], F32, tag="den0", name=f"dn{pair_}{qc_}{hh}")
                nc.vector.tensor_copy(den0, psz[hh][64:65, :])
                rden = small.tile([1, QCH], F32, tag="rden", name=f"rd{pair_}{qc_}{hh}")
                nc.vector.reciprocal_approx_fast(rden, den0)
                rb = small.tile([64, QCH], F32, tag="rb", name=f"rb{pair_}{qc_}{hh}")
                nc.gpsimd.partition_broadcast(rb, rden)
                nc.vector.tensor_mul(
                    zT[pair_][qc_][hh * 64 : (hh + 1) * 64, :],
                    psz[hh][0:64, :],
                    rb,
                )

        # ---- attention main loop ----
        pending = []
        for qc in range(4):
            n_kb = 4 * qc + 4
            for pair in range(2):
                drain_to(need_idx[(qc, pair)])
                psz = [
                    psum_z.tile([65, QCH], F32, tag="z", name=f"psz{pair}{qc}{hh}")
                    for hh in (0, 1)
                ]
                # normalize the previous pair now: its DVE work overlaps this
                # pair's scores/exps, and frees the psz ring for this pair's z
                if pending:
                    flush_normalize(pending.pop(0))
                zq = None  # z work of the previous group (one-group lag)
                for g in range(n_kb // 4):
                    kbs = range(4 * g, 4 * g + 4)
                    pss_g = {}
                    for kb in kbs:
                        dt2 = kb - (n_kb - 4)
                        rel = max(dt2, 0) * 128
                        pss = psum_s.tile(
                            [128, 2 * QCH], F32, tag="s", name=f"pss{pair}{qc}{kb}"
                        )
                        pss_g[kb] = (pss, rel, dt2)
                        for hh in (0, 1):
                            hoff = hh * 64
                            nc.tensor.matmul(
                                pss[:, hh * QCH + rel : (hh + 1) * QCH],
                                lhsT=kT[pair][hoff : hoff + 64, kb * 128 : (kb + 1) * 128],
                                rhs=qT[pair][
                                    hoff : hoff + 64, qc * QCH + rel : (qc + 1) * QCH
                                ],
                                start=True,
                                stop=True,
                                tile_position=(hoff, 0),
                            )
                        pt = pt_pool.tile(
                            [128, 2 * QCH], BF16, tag="pt", name=f"pt{pair}{qc}{kb}"
                        )
                        pss_g[kb] += (pt,)
                        if rel >= 256:
                            for hh in (0, 1):
                                off = hh * QCH + rel
                                nc.scalar.activation(
                                    pt[:, off : hh * QCH + QCH],
                                    pss[:, off : hh * QCH + QCH],
                                    EXP,
                                    scale=0.125,
                                )
                        else:
                            nc.scalar.activation(pt, pss, EXP, scale=0.125)
                        if dt2 >= 0:
                            for hh in (0, 1):
                                off = hh * QCH + rel
                                nc.vector.tensor_mul(
                                    pt[:, off : off + 128], pt[:, off : off + 128], mask_sb
                                )
                    # z of the PREVIOUS group: its exps finished long ago,
                    # and with the filler it pads the PE stream so the next
                    # group's scores never catch the exp spine (phase lag)
                    if zq is not None:
                        for hh in (0, 1):
                            for kb, (pss_, rel_, dt2_, pt_) in zq:
                                nc.tensor.matmul(
                                    psz[hh][:, rel_:QCH],
                                    lhsT=vp[:, 2 * pair + hh, kb, :],
                                    rhs=pt_[:, hh * QCH + rel_ : (hh + 1) * QCH],
                                    start=(kb == 0),
                                    stop=(kb == n_kb - 1),
                                )
                    zq = [(kb, pss_g[kb]) for kb in kbs]
                    # interleave a filler chunklet into the exp-bound slack
                    emit_chunklets(1)
                # final group's z at pair end
                for hh in (0, 1):
                    for kb, (pss_, rel_, dt2_, pt_) in zq:
                        nc.tensor.matmul(
                            psz[hh][:, rel_:QCH],
                            lhsT=vp[:, 2 * pair + hh, kb, :],
                            rhs=pt_[:, hh * QCH + rel_ : (hh + 1) * QCH],
                            start=(kb == 0),
                            stop=(kb == n_kb - 1),
                        )
                pending.append((psz, pair, qc))
                # out-projections deferred into qc3 (PE idle under its exp
                # load): remaining proj(0)/(1) blocks drain at (3,0) end,
                # proj(2) blocks fill (3,1)'s groups
                if qc == 3 and pair == 0:
                    drain_to(34)
        drain_to(len(chunk_queue))
        while pending:
            flush_normalize(pending.pop(0))
        emit_proj(3)

        ost.release()
        small.release()
        pt_pool.release()
        proj.release()
        psum_z.release()
        psum_s.release()
        persist.release()
        const_pool.release()

    nc.compile()
    return nc


_NC_CACHE: list = []


def _get_nc() -> bass.Bass:
    if not _NC_CACHE:
        _NC_CACHE.append(build_bass())
    return _NC_CACHE[0]


def _core_inputs(x, W_Q, W_K, W_V, W_O, c):
    b = c // HPC
    h0 = HPC * (c % HPC)
    wq = np.stack(
        [W_Q[h0 + 2 * p : h0 + 2 * p + 2].transpose(1, 0, 2).reshape(D, 128) for p in range(2)]
    )
    wk = np.stack(
        [W_K[h0 + 2 * p : h0 + 2 * p + 2].transpose(1, 0, 2).reshape(D, 128) for p in range(2)]
    )
    wv = W_V[h0 : h0 + HPC].transpose(1, 0, 2).reshape(D, HPC * Dh)
    wo = np.stack([W_O[h0 + 2 * p : h0 + 2 * p + 2].reshape(128, D) for p in range(2)])
    import ml_dtypes

    bf = ml_dtypes.bfloat16
    return {
        "xt": np.ascontiguousarray(x[b].T.astype(bf)),
        "wq": np.ascontiguousarray(wq.astype(bf)),
        "wk": np.ascontiguousarray(wk.astype(bf)),
        "wv": np.ascontiguousarray(wv.astype(bf)),
        "wo": np.ascontiguousarray(wo.astype(bf)),
    }


def _ensure_ntff_hook():
    """Install the axon NTFF profile hook if the image's antenv lacks it.

    Only needed for trace=True runs (test harness); the grading path
    (kernel()) never calls this.
    """
    try:
        from antenv.axon_hooks import get_axon_ntff_profile_hook  # noqa: F401
        return
    except ImportError:
        pass
    import types

    import antenv

    holder = {"hook": None}
    mod = types.ModuleType("antenv.axon_hooks")
    mod.set_axon_ntff_profile_hook = lambda h: holder.__setitem__("hook", h)
    mod.get_axon_ntff_profile_hook = lambda: holder["hook"]
    sys.modules["antenv.axon_hooks"] = mod
    antenv.axon_hooks = mod
    try:
        if "/root/.axon_site" not in sys.path:
            sys.path.insert(0, "/root/.axon_site")
        from trn_agent_boot.trn_boot import _ntff_profile_via_ctypes

        so = "/opt/axon/libaxon_pjrt.so"
        if os.path.exists(so):
            mod.set_axon_ntff_profile_hook(_ntff_profile_via_ctypes(so))
    except Exception as e:  # degrade to no tracing
        print(f"NTFF hook install failed: {e}", file=sys.stderr)
    # artifact upload needs S3 creds this container may not have
    import concourse.bass_utils as bu

    bu.upload_artifacts = lambda tmpdir: f"local://{tmpdir}"


def _run(inputs: dict, trace: bool = False):
    x = np.asarray(inputs["x"], np.float32)
    W_Q = np.asarray(inputs["W_Q"], np.float32)
    W_K = np.asarray(inputs["W_K"], np.float32)
    W_V = np.asarray(inputs["W_V"], np.float32)
    W_O = np.asarray(inputs["W_O"], np.float32)
    b_O = np.asarray(inputs["b_O"], np.float32)

    if trace:
        _ensure_ntff_hook()
    nc = _get_nc()
    in_maps = [_core_inputs(x, W_Q, W_K, W_V, W_O, c) for c in range(N_CORES)]
    res = run_bass_kernel_spmd(nc, in_maps, core_ids=list(range(N_CORES)), trace=trace)

    out = np.zeros((B, S, D), np.float32)
    for c in range(N_CORES):
        out[c // HPC] += np.asarray(res.results[c]["out"], np.float32)
    out += b_O.sum(axis=0)  # b_O is [H, D]; reference adds sum over heads
    return out, res


def kernel(**inputs) -> np.ndarray:
    # b_Q/b_K/b_V are zero in the reference's setup_inputs; the device
    # kernel folds them out. Guard with an exact fallback just in case.
    for name in ("b_Q", "b_K", "b_V"):
        if name in inputs and np.any(np.asarray(inputs[name])):
            return _kernel_numpy_fallback(**inputs)
    out, _ = _run(inputs)
    if not np.isfinite(out).all():
        # transient device flake (observed rarely); one retry clears it
        out, _ = _run(inputs)
    return out


def _kernel_numpy_fallback(x, W_Q, b_Q, W_K, b_K, W_V, W_O, b_V, b_O):
    x = np.asarray(x, np.float32)
    q = np.einsum("bqm,hmd->bqhd", x, W_Q) + b_Q
    k = np.einsum("bkm,hmd->bkhd", x, W_K) + b_K
    v = np.einsum("bkm,hmd->bkhd", x, W_V) + b_V
    s = np.einsum("bqhd,bkhd->bhqk", q, k) / np.sqrt(np.float32(W_Q.shape[-1]))
    causal = np.tril(np.ones((x.shape[1], x.shape[1]), bool))
    s = np.where(causal, s, np.float32(-1e9))
    s = s - s.max(-1, keepdims=True)
    e = np.exp(s)
    attn = e / e.sum(-1, keepdims=True)
    z = np.einsum("bhqk,bkhd->bqhd", attn, v)
    return np.einsum("bqhd,hdm->bqm", z, W_O) + b_O.sum(0)
